# revision 29
# baseline (speedup 1.0000x reference)
"""Single transformer block on 8 NeuronCores — collective-free, v3.

Sharding: core c = (batch b=c//2, parity p=c%2). Each core receives the FULL
sequence of its batch, permuted to [own-stripe | peer-stripe] order, and
recomputes K and V for all 2048 tokens locally.  Q / attention / c_proj / MLP
cover only the core's 1024 own (striped) tokens.

v3 structure (driven by the TimelineSim cost model):
  - exp() can only run on the ACT engine and is the hard floor of the
    attention phase (~150us); the MLP is the PE floor (~110us).  v3 runs the
    two query-groups (512 own tokens each) OUTER, so after group 0 finishes
    attention, its c_proj/LN2/fc/mproj work is emitted interleaved with
    group 1's attention — the PE-heavy MLP fills the PE slack under group
    1's ACT-bound exp stream.  K^T/Q^T projection passes fill group 0.
  - The residual trunk is carried SCALED BY 64 on-chip (xloc = 64*x host
    side; host divides the output by 64).  Weights are fp8 at x64,
    activations fp8 at true scale (k/q at x8), so every PSUM arrives at
    64x (scores: 64x) truth and all rescales fold into activation-scale
    slots that are free.  x2 is written in place into xloc.
  - fc and mproj run fp8 DoubleRow with a WEIGHT-RESIDUAL second pass
    (W ~= Q(64W) + Q(64W - Q(64W))): half the bf16 cost, and the dominant
    weight-quantization error term vanishes.  c_proj is plain fp8 DR.
  - All biases are free: LN affines fold into following matmuls host-side;
    bv folds into bcp (softmax weights sum to 1); bcp/bmp ride into the
    c_proj/mproj PSUM as rank-1 matmuls; bk/bq fold into K/Q evictions;
    bfc into the gelu activation.
  - Causal masking: a -1920 (= 64 * -30) additive bias matrix accumulated
    onto the diagonal score block by an identity matmul; exp flushes those
    lanes to 0 in fp8.  No DVE mask multiplies.
  - Softmax denominators: V carries a ones column so AV row 64 is the
    denominator; reciprocal + P*(1/d) on DVE, partition-broadcast on the
    (PSUM-blind but otherwise idle) GPSIMD engine.
  - PSUM budget (8 banks): QK 2x[128,2,512] (4) + AV [65,2,512] (2) +
    a single shared [128,512] f32 scratch tag (2) used by every filler
    stage: k/q passes, c_proj, LN2 transposes (f32), fc, mproj.
"""

import math
from collections import deque
from contextlib import ExitStack

import numpy as np
import ml_dtypes

import concourse.bacc as bacc
import concourse.bass as bass
import concourse.mybir as mybir
import concourse.tile as tile
from concourse.masks import make_identity

F32 = mybir.dt.float32
BF16 = mybir.dt.bfloat16
F8 = mybir.dt.float8e4
AF = mybir.ActivationFunctionType
ALU = mybir.AluOpType
DR = mybir.MatmulPerfMode.DoubleRow

EPS = 1e-5
TRUNK = 64.0       # residual trunk scale carried on-chip
WS = 64.0          # fp8 weight scale
KQS = 8.0          # k/q fp8 storage scale (scores come out at 64x)
NEGB = -30.0 * 64  # additive mask bias at score-PSUM scale


class Cfg:
    def __init__(self, B=4, S=2048, D=1024, H=16, F=4096, n_cores=8, BS=128):
        self.B, self.S, self.D, self.H, self.F = B, S, D, H, F
        self.n_cores = n_cores
        assert n_cores == 2 * B
        self.HD = D // H
        assert self.HD == 64
        self.T = S // 2            # tokens owned per core
        self.KT = S // 128         # token 128-tiles, full sequence
        self.TB = self.T // 128    # token 128-tiles, local
        self.DC = D // 128         # contraction chunks over D
        self.KC = S // 128         # key 128-chunks over full sequence
        self.GB = F // 128         # MLP hidden 128-blocks
        self.HPB = 128 // self.HD  # heads per 128-feature block (=2)
        self.BS = BS               # stripe block (q-slot) size
        self.SLOTS = self.T // self.BS
        self.SPG = 512 // self.BS  # slots per 512-wide attention group
        self.KCH = self.KC // 2    # chunks per parity half
        self.CPB = self.BS // 128  # key chunks per stripe block


def build(cfg: Cfg):
    c = cfg
    nc = bacc.Bacc(None, target_bir_lowering=False)
    inv_w = 1.0 / WS

    # ---------------- I/O ----------------
    x_in = nc.dram_tensor("x", [c.S, c.D], F32, kind="ExternalInput")
    w_attn = nc.dram_tensor("w_attn", [c.D, 3 * c.D], F8, kind="ExternalInput")
    w_cproj = nc.dram_tensor("w_cproj", [c.D, c.D], F8, kind="ExternalInput")
    w_fc = nc.dram_tensor("w_fc", [c.D, c.F], F8, kind="ExternalInput")
    r_fc = nc.dram_tensor("r_fc", [c.D, c.F], F8, kind="ExternalInput")
    w_mproj = nc.dram_tensor("w_mproj", [c.F, c.D], F8, kind="ExternalInput")
    r_mproj = nc.dram_tensor("r_mproj", [c.F, c.D], F8, kind="ExternalInput")
    battn_qk_in = nc.dram_tensor("battn_qk", [128, 2 * c.DC], F32,
                                 kind="ExternalInput")
    bcp_in = nc.dram_tensor("bcp", [1, c.D], BF16, kind="ExternalInput")
    bmp_in = nc.dram_tensor("bmp", [1, c.D], BF16, kind="ExternalInput")
    bfc_in = nc.dram_tensor("bfc", [128, c.GB], F32, kind="ExternalInput")
    qidx_in = nc.dram_tensor("qidx", [1, c.T], F32, kind="ExternalInput")
    kofs_in = nc.dram_tensor("kofs", [128, c.KC], F32, kind="ExternalInput")
    y_out = nc.dram_tensor("y", [c.T, c.D], F32, kind="ExternalOutput")

    def bcast(dram, p=128):
        return bass.AP(tensor=dram, offset=0, ap=[[0, p], [1, dram.shape[1]]])

    with tile.TileContext(nc) as tc, ExitStack() as es:
        gconst = es.enter_context(tc.tile_pool(name="gconst", bufs=1))
        identb = gconst.tile([128, 128], BF16, name="identb")
        make_identity(nc, identb[:])
        identf = gconst.tile([128, 128], F32, name="identf")
        make_identity(nc, identf[:])
        eps_t = gconst.tile([128, 1], F32)
        nc.vector.memset(eps_t[:], EPS)
        onesb = gconst.tile([1, 128], BF16, name="onesb")
        nc.vector.memset(onesb[:], 1.0)
        bcp_row = gconst.tile([1, c.D], BF16, name="bcp_row")
        nc.sync.dma_start(out=bcp_row[:], in_=bcp_in[:, :])
        bmp_row = gconst.tile([1, c.D], BF16, name="bmp_row")
        nc.sync.dma_start(out=bmp_row[:], in_=bmp_in[:, :])
        battn_qk = gconst.tile([128, 2 * c.DC], F32, name="battn")
        nc.sync.dma_start(out=battn_qk[:], in_=battn_qk_in[:, :])
        bfc = gconst.tile([128, c.GB], F32, name="bfc")
        nc.sync.dma_start(out=bfc[:], in_=bfc_in[:, :])
        kofs = gconst.tile([128, c.KC], F32, name="kofs")
        nc.sync.dma_start(out=kofs[:], in_=kofs_in[:, :])
        masks = gconst.tile([128, c.KC, c.BS], BF16, name="masks")

        # ------------- persistent pools -------------
        # (pools reserve their footprint for their whole open lifetime, so
        # the c_proj/mproj/MLP pools open only after phase A)
        xlp = es.enter_context(tc.tile_pool(name="xloc", bufs=1, side="left"))
        kvqp = es.enter_context(tc.tile_pool(name="kvq", bufs=1,
                                             side="right"))

        xloc = []
        for tb in range(c.TB):
            t = xlp.tile([128, c.D], F32, tag=f"x{tb}", name=f"x{tb}")
            nc.sync.dma_start(out=t[:], in_=x_in[tb * 128:(tb + 1) * 128, :])
            xloc.append(t)

        ktp = kvqp.tile([128, c.DC, c.S], F8, name="ktp")
        vtt = kvqp.tile([128, 2, c.KCH, c.H, 65], F8, name="vtt")
        qtp = kvqp.tile([128, c.DC, c.T], F8, name="qtp")
        nc.vector.memset(vtt[:, :, :, :, 64:65], 1.0)

        # ================= phase A-head: LN1 + QKV-V =================
        es_ht = ExitStack()
        htp = es_ht.enter_context(tc.tile_pool(name="htp", bufs=1,
                                       side="right"))
        ht = htp.tile([128, c.DC, c.S], F8, name="ht")

        es_wa = ExitStack()
        wap = es_wa.enter_context(tc.tile_pool(name="wa", bufs=2,
                                       side="right"))
        wk = wap.tile([128, c.DC, c.D], F8, tag="wa", name="wk")
        nc.scalar.dma_start(
            out=wk[:],
            in_=w_attn[:, c.D:2 * c.D].rearrange("(i p) f -> p i f", p=128))
        wq = wap.tile([128, c.DC, c.D], F8, tag="wa", name="wq")
        nc.scalar.dma_start(
            out=wq[:],
            in_=w_attn[:, 0:c.D].rearrange("(i p) f -> p i f", p=128))

        NI = c.DC // 2  # DoubleRow contraction steps over D

        with (
            tc.tile_pool(name="xs", bufs=4) as xsp,
            tc.tile_pool(name="lnp", bufs=4) as lnp,
            tc.tile_pool(name="wvp", bufs=1, side="right") as wvp,
            tc.tile_pool(name="ps_tr", bufs=2, space="PSUM") as ps_tr,
            tc.tile_pool(name="ps_v", bufs=2, space="PSUM") as psv,
        ):
            wv = wvp.tile([128, c.DC, c.D], F8, name="wv")
            nc.scalar.dma_start(
                out=wv[:],
                in_=w_attn[:, 2 * c.D:3 * c.D].rearrange(
                    "(i p) f -> p i f", p=128))
            def v_chunk(g):
                for tb in range(4 * g, 4 * g + 4):
                    tbs = slice(tb * 128, (tb + 1) * 128)
                    pss = [psv.tile([128, 512], F32, tag="ps", name=f"psv{q}")
                           for q in range(2)]
                    for i in range(NI):
                        for vh in range(2):
                            nc.tensor.matmul(
                                pss[vh][:], ht[:, 2 * i:2 * i + 2, tbs],
                                wv[:, 2 * i:2 * i + 2,
                                   vh * 512:(vh + 1) * 512],
                                start=(i == 0), stop=(i == NI - 1),
                                perf_mode=DR)
                    for vh in range(2):
                        dst = vtt[:, tb // c.KCH, tb % c.KCH,
                                  vh * 8:(vh + 1) * 8, 0:64]
                        nc.scalar.activation(dst, pss[vh][:], AF.Identity,
                                             scale=inv_w)

            for tb in range(c.KT):
                if tb < c.TB:
                    src = xloc[tb]
                else:
                    src = xsp.tile([128, c.D], F32, tag="xs")
                    nc.sync.dma_start(
                        out=src[:], in_=x_in[tb * 128:(tb + 1) * 128, :])
                st = lnp.tile([128, 2, 6], F32, tag="ast")
                for sg in range(2):
                    nc.vector.bn_stats(
                        out=st[:, sg, :], in_=src[:, sg * 512:(sg + 1) * 512])
                mv = lnp.tile([128, 2], F32, tag="amv")
                nc.vector.bn_aggr(out=mv[:], in_=st[:])
                sd = lnp.tile([128, 1], F32, tag="asd")
                nc.scalar.activation(sd[:], mv[:, 1:2], AF.Sqrt,
                                     bias=eps_t[:, 0:1])
                rs = lnp.tile([128, 1], F32, tag="ars")
                nc.vector.reciprocal(rs[:], sd[:])
                nrm = lnp.tile([128, c.D], BF16, tag="an")
                nc.vector.tensor_scalar(
                    out=nrm[:], in0=src[:], scalar1=mv[:, 0:1],
                    scalar2=rs[:, 0:1], op0=ALU.subtract, op1=ALU.mult)
                for i2 in range(2):
                    pt = ps_tr.tile([128, 512], BF16, tag="atr")
                    for j in range(4):
                        ch = 4 * i2 + j
                        nc.tensor.matmul(
                            pt[:, j * 128:(j + 1) * 128],
                            nrm[:, ch * 128:(ch + 1) * 128], identb[:],
                            is_transpose=True, start=(j == 0), stop=(j == 3))
                    dst = ht[:, 4 * i2:4 * i2 + 4, tb * 128:(tb + 1) * 128]
                    if (tb + i2) % 2 == 0:
                        nc.vector.tensor_copy(dst, pt[:])
                    else:
                        nc.scalar.activation(dst, pt[:], AF.Identity)
                if tb % 4 == 3:
                    v_chunk(tb // 4)

        # late-opening persistent pools (phase A needed the headroom)
        wcp = es.enter_context(tc.tile_pool(name="wc", bufs=1, side="left"))
        atp = es.enter_context(tc.tile_pool(name="atp", bufs=1, side="left"))
        mgp = es.enter_context(tc.tile_pool(name="mgp", bufs=1, side="left"))
        wmp = es.enter_context(tc.tile_pool(name="wm", bufs=1, side="left"))

        # c_proj weights prefetch on the sync queue; mproj slabs stream
        # per (half, fh) later (SBUF is too tight to keep them resident)
        wc = wcp.tile([128, c.DC, c.D], F8, name="wc")
        nc.sync.dma_start(
            out=wc[:], in_=w_cproj[:, :].rearrange("(i p) f -> p i f", p=128))

        def load_mslab(fh):
            wm = wmp.tile([128, c.GB, 512], F8, tag="wm", name=f"wm{fh}")
            nc.sync.dma_start(
                out=wm[:],
                in_=w_mproj[:, fh * 512:(fh + 1) * 512].rearrange(
                    "(g p) f -> p g f", p=128))
            rm = wmp.tile([128, c.GB, 512], F8, tag="rm", name=f"rm{fh}")
            nc.sync.dma_start(
                out=rm[:],
                in_=r_mproj[:, fh * 512:(fh + 1) * 512].rearrange(
                    "(g p) f -> p g f", p=128))
            return wm, rm

        # mask bias matrices (GPSIMD; SBUF only) from a short-lived qidx tile
        with tc.tile_pool(name="qip", bufs=1) as qip:
            qidx = qip.tile([128, c.T], F32, name="qidx")
            nc.sync.dma_start(out=qidx[:], in_=bcast(qidx_in))
            for kc in range(c.KC):
                s_min = (kc % c.KCH) // c.CPB
                qsl = slice(s_min * c.BS, (s_min + 1) * c.BS)
                nc.gpsimd.tensor_scalar(
                    out=masks[:, kc, :], in0=qidx[:, qsl],
                    scalar1=kofs[:, kc:kc + 1], scalar2=NEGB,
                    op0=ALU.is_lt, op1=ALU.mult)

        # per-half reused activity tiles (allocated only now — phase A
        # needs the SBUF headroom)
        at2 = [atp.tile([128, c.DC, 512], F8, name=f"at{i}")
               for i in range(2)]
        mt = mgp.tile([128, c.DC, 512], F8, name="mt")
        gt = mgp.tile([128, c.GB, 512], F8, name="gt")

        # ============ interleaved region: K/Q + attention + MLP ============
        with (
            tc.tile_pool(name="pt", bufs=6) as ptp,
            tc.tile_pool(name="rec", bufs=2) as recp,
            tc.tile_pool(name="lnp2", bufs=2) as lnp2,
            tc.tile_pool(name="yout", bufs=2) as yop,
            tc.tile_pool(name="ps_qk", bufs=2, space="PSUM") as psqk,
            tc.tile_pool(name="ps_o", bufs=1, space="PSUM") as pso,
            tc.tile_pool(name="misc", bufs=2, space="PSUM") as miscp,
        ):
            def mtile():
                return miscp.tile([128, 512], F32, tag="m", name="m")

            # ---- filler pieces ----
            def kq_piece(m):
                def emit():
                    msl = slice(m * 128, (m + 1) * 128)
                    for th in range(4):
                        ps = mtile()
                        for i in range(NI):
                            nc.tensor.matmul(
                                ps[:], wk[:, 2 * i:2 * i + 2, msl],
                                ht[:, 2 * i:2 * i + 2,
                                   th * 512:(th + 1) * 512],
                                start=(i == 0), stop=(i == NI - 1),
                                perf_mode=DR)
                        if th % 2 == 0:
                            nc.vector.tensor_scalar(
                                out=ktp[:, m, th * 512:(th + 1) * 512],
                                in0=ps[:], scalar1=1.0 / (WS / KQS),
                                scalar2=battn_qk[:, c.DC + m:c.DC + m + 1],
                                op0=ALU.mult, op1=ALU.add)
                        else:
                            nc.scalar.activation(
                                ktp[:, m, th * 512:(th + 1) * 512], ps[:],
                                AF.Identity,
                                bias=battn_qk[:, c.DC + m:c.DC + m + 1],
                                scale=1.0 / (WS / KQS))
                    for th in range(2):
                        ps = mtile()
                        for i in range(NI):
                            nc.tensor.matmul(
                                ps[:], wq[:, 2 * i:2 * i + 2, msl],
                                ht[:, 2 * i:2 * i + 2,
                                   th * 512:(th + 1) * 512],
                                start=(i == 0), stop=(i == NI - 1),
                                perf_mode=DR)
                        if th % 2 == 1:
                            nc.vector.tensor_scalar(
                                out=qtp[:, m, th * 512:(th + 1) * 512],
                                in0=ps[:], scalar1=1.0 / (WS / KQS),
                                scalar2=battn_qk[:, m:m + 1],
                                op0=ALU.mult, op1=ALU.add)
                        else:
                            nc.scalar.activation(
                                qtp[:, m, th * 512:(th + 1) * 512], ps[:],
                                AF.Identity, bias=battn_qk[:, m:m + 1],
                                scale=1.0 / (WS / KQS))
                return emit

            stats_box = {}

            def cproj_piece(tb, gi):
                """c_proj + residual for one tile, plus LN2 bn-stats.  The
                sqrt lives in ln2_piece so the 4 tiles share ONE ACT sqrt
                (each Exp<->Sqrt table switch costs 1283ns)."""
                def emit():
                    tbl = slice((tb - 4 * gi) * 128, (tb - 4 * gi + 1) * 128)
                    NC = c.DC // 2
                    for fh in range(2):
                        fsl = slice(fh * 512, (fh + 1) * 512)
                        ps = mtile()
                        for i in range(NC):
                            nc.tensor.matmul(
                                ps[:], at2[gi][:, 2 * i:2 * i + 2, tbl],
                                wc[:, 2 * i:2 * i + 2, fsl],
                                start=(i == 0), stop=False, perf_mode=DR)
                        nc.tensor.matmul(ps[:], onesb[:], bcp_row[0:1, fsl],
                                         start=False, stop=True)
                        # x2 written in place into the residual trunk
                        nc.vector.tensor_add(xloc[tb][:, fsl], ps[:],
                                             xloc[tb][:, fsl])
                    if tb % 4 == 0:
                        stats_box["mv"] = lnp2.tile([128, 4, 2], F32,
                                                    tag="dmv", name="dmv")
                    src = xloc[tb]
                    st = lnp2.tile([128, 2, 6], F32, tag="dst")
                    for sg in range(2):
                        nc.vector.bn_stats(
                            out=st[:, sg, :],
                            in_=src[:, sg * 512:(sg + 1) * 512])
                    nc.vector.bn_aggr(out=stats_box["mv"][:, tb % 4, :],
                                      in_=st[:])
                return emit

            def ln2_piece(gi):
                """one batched sqrt for the half's 4 tiles, then normalize +
                transpose + evict each tile into mt."""
                def emit():
                    mv = stats_box["mv"]
                    sd = lnp2.tile([128, 4], F32, tag="dsd")
                    nc.scalar.activation(sd[:], mv[:, :, 1], AF.Sqrt,
                                         bias=eps_t[:, 0:1])
                    rs = lnp2.tile([128, 4], F32, tag="drs")
                    nc.vector.reciprocal(rs[:], sd[:])
                    for tb in range(4 * gi, 4 * gi + 4):
                        tbl = slice((tb - 4 * gi) * 128,
                                    (tb - 4 * gi + 1) * 128)
                        i4 = tb % 4
                        nrm = lnp2.tile([128, c.D], F32, tag="dn")
                        nc.vector.tensor_scalar(
                            out=nrm[:], in0=xloc[tb][:],
                            scalar1=mv[:, i4, 0:1], scalar2=rs[:, i4:i4 + 1],
                            op0=ALU.subtract, op1=ALU.mult)
                        for i2 in range(2):
                            pt = mtile()
                            for j in range(4):
                                ch = 4 * i2 + j
                                nc.tensor.matmul(
                                    pt[:, j * 128:(j + 1) * 128],
                                    nrm[:, ch * 128:(ch + 1) * 128],
                                    identf[:],
                                    is_transpose=True, start=(j == 0),
                                    stop=(j == 3))
                            nc.vector.tensor_copy(
                                mt[:, 4 * i2:4 * i2 + 4, tbl], pt[:])
                return emit

            wfp_box = {}
            es_fc = ExitStack()

            def fc_piece(gb0, gi):
                # 16 hidden-blocks per piece: the gelus cluster so the
                # Exp<->Gelu ACT table switch is paid twice per piece, not
                # per 4-block group
                def emit():
                    if "p" not in wfp_box:
                        wfp_box["p"] = es_fc.enter_context(
                            tc.tile_pool(name="wf", bufs=2, side="left"))
                    wfp = wfp_box["p"]
                    for gb in range(gb0, gb0 + 16):
                        if gb % 4 == 0:
                            j = gb // 4
                            wf = wfp.tile([128, c.DC, 512], F8, tag="wf",
                                          name=f"wf{gi}_{gb}")
                            nc.scalar.dma_start(
                                out=wf[:],
                                in_=w_fc[:, j * 512:(j + 1) * 512].rearrange(
                                    "(i p) f -> p i f", p=128))
                            rf = wfp.tile([128, c.DC, 512], F8, tag="rf",
                                          name=f"rf{gi}_{gb}")
                            nc.scalar.dma_start(
                                out=rf[:],
                                in_=r_fc[:, j * 512:(j + 1) * 512].rearrange(
                                    "(i p) f -> p i f", p=128))
                            emit.wf, emit.rf = wf, rf
                        wf, rf = emit.wf, emit.rf
                        gl = (gb % 4) * 128
                        ps = mtile()
                        for wslab, first in ((wf, True), (rf, False)):
                            for i in range(NI):
                                nc.tensor.matmul(
                                    ps[:],
                                    wslab[:, 2 * i:2 * i + 2, gl:gl + 128],
                                    mt[:, 2 * i:2 * i + 2, :],
                                    start=(first and i == 0),
                                    stop=(not first and i == NI - 1),
                                    perf_mode=DR)
                        nc.scalar.activation(
                            gt[:, gb, :], ps[:], AF.Gelu_apprx_tanh,
                            bias=bfc[:, gb:gb + 1], scale=inv_w)
                return emit

            yo_tiles = {}
            mslab = {}

            def mslab_prefetch(fh):
                # issue the slab DMA well before the first mproj piece needs
                # it (the DMA still waits on the previous slab's last reader)
                def emit():
                    mslab[fh] = load_mslab(fh)
                return emit

            def mproj_piece(tb, gi, fh):
                def emit():
                    tbl = slice((tb - 4 * gi) * 128, (tb - 4 * gi + 1) * 128)
                    NG = c.GB // 2
                    wm, rm = mslab[fh]
                    yo = yop.tile([128, 512], F32, tag="yo",
                                  name=f"yo{tb}_{fh}")
                    fsl = slice(fh * 512, (fh + 1) * 512)
                    ps = mtile()
                    for wslab, first in ((wm, True), (rm, False)):
                        for g in range(NG):
                            nc.tensor.matmul(
                                ps[:], gt[:, 2 * g:2 * g + 2, tbl],
                                wslab[:, 2 * g:2 * g + 2, :],
                                start=(first and g == 0),
                                stop=False, perf_mode=DR)
                    nc.tensor.matmul(ps[:], onesb[:], bmp_row[0:1, fsl],
                                     start=False, stop=True)
                    nc.vector.tensor_add(yo[:], ps[:], xloc[tb][:, fsl])
                    nc.sync.dma_start(
                        out=y_out[tb * 128:(tb + 1) * 128, fsl], in_=yo[:])
                return emit

            # ---- attention for one (jj, gi) block ----
            groups = [list(range(c.SPG * gi, c.SPG * (gi + 1)))
                      for gi in range(c.SLOTS // c.SPG)]

            def attention(jj, gi):
                # QK/exp run LAG plain-steps ahead of AV so the AV of the
                # next block never stalls the in-order PE queue on the
                # previous block's softmax-normalize reads of `pos`.
                LAG = 2
                g = groups[gi]
                s0, s3 = g[0], g[-1]
                n_loc = (s3 + 1) * c.CPB
                pos = pso.tile([65, 2, 512], F32, tag="po", name="po")
                pending = deque()

                def emit_av(item):
                    loc, hp, h, lo, pt, w = item
                    nc.tensor.matmul(
                        pos[:, hp, (lo - s0) * c.BS:512],
                        vtt[:, :, loc, h, :], pt[:, :, 0:w],
                        start=(loc == 0), stop=(loc == n_loc - 1),
                        perf_mode=DR)

                for loc in range(n_loc):
                    lo = max(s0, loc // c.CPB)
                    w = (s3 - lo + 1) * c.BS
                    qsl = slice(lo * c.BS, (s3 + 1) * c.BS)
                    diag = loc // c.CPB >= s0
                    for hp in range(c.HPB):
                        h = c.HPB * jj + hp
                        base = hp * 64
                        ps2 = psqk.tile([128, 2, 512], F32, tag="qk")
                        pt = ptp.tile([128, 2, 512], F8, tag="pt")
                        for ix in range(2):
                            kc = loc + ix * c.KCH
                            nc.tensor.matmul(
                                ps2[:, ix, 0:w],
                                ktp[base:base + 64, jj,
                                    kc * 128:(kc + 1) * 128],
                                qtp[base:base + 64, jj, qsl],
                                start=True, stop=not diag)
                            if diag:
                                nc.tensor.matmul(
                                    ps2[:, ix, 0:c.BS], identb[:],
                                    masks[:, kc, :],
                                    start=False, stop=True)
                        nc.scalar.activation(pt[:, :, 0:w], ps2[:, :, 0:w],
                                             AF.Exp, scale=1.0 / (KQS * KQS))
                        pending.append((loc, hp, h, lo, pt, w))
                        while len(pending) > LAG * c.HPB:
                            emit_av(pending.popleft())
                while pending:
                    emit_av(pending.popleft())
                for hp in range(c.HPB):
                    base = hp * 64
                    rec = recp.tile([1, 512], F32, tag="rec")
                    nc.vector.reciprocal(rec[:], pos[64:65, hp, :])
                    bcs = recp.tile([64, 512], F32, tag="bcs")
                    nc.gpsimd.partition_broadcast(bcs[:], rec[:])
                    nc.vector.tensor_tensor(
                        out=at2[gi][base:base + 64, jj, :],
                        in0=pos[0:64, hp, :], in1=bcs[:], op=ALU.mult)

            # ---- the interleaved emission ----
            filler = deque(kq_piece(m) for m in range(c.DC))
            for gi in range(2):
                for jj in range(c.DC):
                    # keep the K/Q chunk needed by the NEXT jj ahead of
                    # its attention block during gi=0
                    while (gi == 0 and filler
                           and len(filler) > (c.DC - 2 - jj)):
                        filler.popleft()()
                    attention(jj, gi)
                    if gi == 1:
                        for _ in range(3):
                            if filler:
                                filler.popleft()()
                while gi == 0 and filler:
                    filler.popleft()()
                if gi == 0:
                    es_wa.close()
                    es_ht.close()
                # queue this half's MLP chain as filler for the next half.
                # mproj fh-order alternates per half so the still-resident
                # slab pair is reused (3 slab loads total instead of 4)
                for tb in range(4 * gi, 4 * gi + 4):
                    filler.append(cproj_piece(tb, gi))
                filler.append(ln2_piece(gi))
                fhs = (0, 1) if gi == 0 else (1, 0)
                filler.append(mslab_prefetch(fhs[0]) if gi == 0
                              else fc_piece(0, gi))
                filler.append(fc_piece(0, gi) if gi == 0
                              else fc_piece(16, gi))
                if gi == 0:
                    filler.append(fc_piece(16, gi))
                for tb in range(4 * gi, 4 * gi + 4):
                    filler.append(mproj_piece(tb, gi, fhs[0]))
                filler.append(mslab_prefetch(fhs[1]))
                for tb in range(4 * gi, 4 * gi + 4):
                    filler.append(mproj_piece(tb, gi, fhs[1]))
            while filler:
                filler.popleft()()
            es_fc.close()

    nc.compile()
    return nc


def core_rows(cfg, half):
    """absolute sequence rows owned by a core with parity half"""
    c = cfg
    loc = np.arange(c.T)
    return (2 * (loc // c.BS) + half) * c.BS + loc % c.BS


def make_core_inputs(cfg: Cfg, x, ln1_w, ln1_b, W_attn, b_attn, W_cproj,
                     b_cproj, ln2_w, ln2_b, W_fc, b_fc, W_mproj, b_mproj):
    """Split full inputs into one in_map per core."""
    c = cfg
    f32 = np.float32
    f8 = ml_dtypes.float8_e4m3fn
    bf = ml_dtypes.bfloat16

    def q8w(w):
        return np.ascontiguousarray(np.asarray(w, f32) * WS).astype(f8)

    # fold LN1 affine + query scale into W_attn / b_attn
    ln1_w = np.asarray(ln1_w, f32)
    ln1_b = np.asarray(ln1_b, f32)
    Wa = np.asarray(W_attn, f32) * ln1_w[:, None]
    ba = np.asarray(b_attn, f32) + ln1_b @ np.asarray(W_attn, f32)
    qs = 1.0 / math.sqrt(c.HD)
    Wa = Wa.copy()
    Wa[:, :c.D] *= qs
    ba = ba.copy()
    ba[:c.D] *= qs

    # fold LN2 affine into W_fc / b_fc
    ln2_w = np.asarray(ln2_w, f32)
    ln2_b = np.asarray(ln2_b, f32)
    Wf = np.asarray(W_fc, f32) * ln2_w[:, None]
    bf_fold = np.asarray(b_fc, f32) + ln2_b @ np.asarray(W_fc, f32)

    # fold the V bias through the softmax (weights sum to 1): it becomes a
    # constant shift of the attention output -> bv @ W_cproj joins b_cproj
    bv = ba[2 * c.D:3 * c.D]
    bcp_tot = np.asarray(b_cproj, f32) + bv @ np.asarray(W_cproj, f32)

    Wf8 = q8w(Wf)
    Rf8 = np.ascontiguousarray(
        np.asarray(Wf, f32) * WS - np.asarray(Wf8, f32)).astype(f8)
    Wm8 = q8w(W_mproj)
    Rm8 = np.ascontiguousarray(
        np.asarray(W_mproj, f32) * WS - np.asarray(Wm8, f32)).astype(f8)

    shared = {
        "w_attn": q8w(Wa),
        "w_cproj": q8w(W_cproj),
        "w_fc": Wf8,
        "r_fc": Rf8,
        "w_mproj": Wm8,
        "r_mproj": Rm8,
        "bcp": np.ascontiguousarray(
            TRUNK * bcp_tot).astype(bf).reshape(1, c.D),
        "bmp": np.ascontiguousarray(
            TRUNK * np.asarray(b_mproj, f32)).astype(bf).reshape(1, c.D),
        "bfc": np.ascontiguousarray(bf_fold.reshape(c.GB, 128).T),
        # k/q evictions: out = psum/(WS/KQS) + KQS*bias
        "battn_qk": np.ascontiguousarray(
            KQS * ba[:2 * c.D].reshape(2 * c.DC, 128).T),
    }

    x = np.asarray(x, f32)
    in_maps = []
    for core in range(c.n_cores):
        b, half = core // 2, core % 2
        own = core_rows(c, half)
        peer = core_rows(c, 1 - half)
        perm = np.concatenate([own, peer])
        m = dict(shared)
        m["x"] = np.ascontiguousarray(TRUNK * x[b][perm])
        m["qidx"] = own.astype(f32).reshape(1, c.T)
        kofs = np.empty((128, c.KC), f32)
        for kc in range(c.KC):
            kofs[:, kc] = perm[kc * 128 + np.arange(128)]
        m["kofs"] = kofs
        in_maps.append(m)
    return in_maps


_NC_CACHE = {}


def get_nc(cfg: Cfg):
    key = (cfg.B, cfg.S, cfg.D, cfg.H, cfg.F, cfg.BS)
    if key not in _NC_CACHE:
        _NC_CACHE[key] = build(cfg)
    return _NC_CACHE[key]


def kernel(**inputs) -> np.ndarray:
    from concourse.bass_utils import run_bass_kernel_spmd

    cfg = Cfg()
    nc = get_nc(cfg)
    in_maps = make_core_inputs(cfg, **inputs)
    res = run_bass_kernel_spmd(nc, in_maps, core_ids=list(range(cfg.n_cores)))
    B, S, D = cfg.B, cfg.S, cfg.D
    out = np.empty((B, S, D), np.float32)
    inv_trunk = 1.0 / TRUNK
    for core in range(cfg.n_cores):
        b, half = core // 2, core % 2
        out[b, core_rows(cfg, half), :] = inv_trunk * res.results[core]["y"]
    return out


# revision 30
# speedup vs baseline: 1.0110x; 1.0110x over previous
"""Single transformer block on 8 NeuronCores — collective-free, v3.

Sharding: core c = (batch b=c//2, parity p=c%2). Each core receives the FULL
sequence of its batch, permuted to [own-stripe | peer-stripe] order, and
recomputes K and V for all 2048 tokens locally.  Q / attention / c_proj / MLP
cover only the core's 1024 own (striped) tokens.

v3 structure (driven by the TimelineSim cost model):
  - exp() can only run on the ACT engine and is the hard floor of the
    attention phase (~150us); the MLP is the PE floor (~110us).  v3 runs the
    two query-groups (512 own tokens each) OUTER, so after group 0 finishes
    attention, its c_proj/LN2/fc/mproj work is emitted interleaved with
    group 1's attention — the PE-heavy MLP fills the PE slack under group
    1's ACT-bound exp stream.  K^T/Q^T projection passes fill group 0.
  - The residual trunk is carried SCALED BY 64 on-chip (xloc = 64*x host
    side; host divides the output by 64).  Weights are fp8 at x64,
    activations fp8 at true scale (k/q at x8), so every PSUM arrives at
    64x (scores: 64x) truth and all rescales fold into activation-scale
    slots that are free.  x2 is written in place into xloc.
  - fc and mproj run fp8 DoubleRow with a WEIGHT-RESIDUAL second pass
    (W ~= Q(64W) + Q(64W - Q(64W))): half the bf16 cost, and the dominant
    weight-quantization error term vanishes.  c_proj is plain fp8 DR.
  - All biases are free: LN affines fold into following matmuls host-side;
    bv folds into bcp (softmax weights sum to 1); bcp/bmp ride into the
    c_proj/mproj PSUM as rank-1 matmuls; bk/bq fold into K/Q evictions;
    bfc into the gelu activation.
  - Causal masking: a -1920 (= 64 * -30) additive bias matrix accumulated
    onto the diagonal score block by an identity matmul; exp flushes those
    lanes to 0 in fp8.  No DVE mask multiplies.
  - Softmax denominators: V carries a ones column so AV row 64 is the
    denominator; reciprocal + P*(1/d) on DVE, partition-broadcast on the
    (PSUM-blind but otherwise idle) GPSIMD engine.
  - PSUM budget (8 banks): QK 2x[128,2,512] (4) + AV [65,2,512] (2) +
    a single shared [128,512] f32 scratch tag (2) used by every filler
    stage: k/q passes, c_proj, LN2 transposes (f32), fc, mproj.
"""

import math
from collections import deque
from contextlib import ExitStack

import numpy as np
import ml_dtypes

import concourse.bacc as bacc
import concourse.bass as bass
import concourse.mybir as mybir
import concourse.tile as tile
from concourse.masks import make_identity

F32 = mybir.dt.float32
BF16 = mybir.dt.bfloat16
F8 = mybir.dt.float8e4
AF = mybir.ActivationFunctionType
ALU = mybir.AluOpType
DR = mybir.MatmulPerfMode.DoubleRow

EPS = 1e-5
TRUNK = 64.0       # residual trunk scale carried on-chip
WS = 64.0          # fp8 weight scale
KQS = 8.0          # k/q fp8 storage scale (scores come out at 64x)
NEGB = -30.0 * 64  # additive mask bias at score-PSUM scale


class Cfg:
    def __init__(self, B=4, S=2048, D=1024, H=16, F=4096, n_cores=8, BS=128):
        self.B, self.S, self.D, self.H, self.F = B, S, D, H, F
        self.n_cores = n_cores
        assert n_cores == 2 * B
        self.HD = D // H
        assert self.HD == 64
        self.T = S // 2            # tokens owned per core
        self.KT = S // 128         # token 128-tiles, full sequence
        self.TB = self.T // 128    # token 128-tiles, local
        self.DC = D // 128         # contraction chunks over D
        self.KC = S // 128         # key 128-chunks over full sequence
        self.GB = F // 128         # MLP hidden 128-blocks
        self.HPB = 128 // self.HD  # heads per 128-feature block (=2)
        self.BS = BS               # stripe block (q-slot) size
        self.SLOTS = self.T // self.BS
        self.SPG = 512 // self.BS  # slots per 512-wide attention group
        self.KCH = self.KC // 2    # chunks per parity half
        self.CPB = self.BS // 128  # key chunks per stripe block


def build(cfg: Cfg):
    c = cfg
    nc = bacc.Bacc(None, target_bir_lowering=False)
    inv_w = 1.0 / WS

    # ---------------- I/O ----------------
    x_in = nc.dram_tensor("x", [c.S, c.D], F32, kind="ExternalInput")
    w_attn = nc.dram_tensor("w_attn", [c.D, 3 * c.D], F8, kind="ExternalInput")
    w_cproj = nc.dram_tensor("w_cproj", [c.D, c.D], F8, kind="ExternalInput")
    w_fc = nc.dram_tensor("w_fc", [c.D, c.F], F8, kind="ExternalInput")
    r_fc = nc.dram_tensor("r_fc", [c.D, c.F], F8, kind="ExternalInput")
    w_mproj = nc.dram_tensor("w_mproj", [c.F, c.D], F8, kind="ExternalInput")
    r_mproj = nc.dram_tensor("r_mproj", [c.F, c.D], F8, kind="ExternalInput")
    battn_qk_in = nc.dram_tensor("battn_qk", [128, 2 * c.DC], F32,
                                 kind="ExternalInput")
    bcp_in = nc.dram_tensor("bcp", [1, c.D], BF16, kind="ExternalInput")
    bmp_in = nc.dram_tensor("bmp", [1, c.D], BF16, kind="ExternalInput")
    bfc_in = nc.dram_tensor("bfc", [128, c.GB], F32, kind="ExternalInput")
    qidx_in = nc.dram_tensor("qidx", [1, c.T], F32, kind="ExternalInput")
    kofs_in = nc.dram_tensor("kofs", [128, c.KC], F32, kind="ExternalInput")
    y_out = nc.dram_tensor("y", [c.T, c.D], F32, kind="ExternalOutput")

    def bcast(dram, p=128):
        return bass.AP(tensor=dram, offset=0, ap=[[0, p], [1, dram.shape[1]]])

    with tile.TileContext(nc) as tc, ExitStack() as es:
        gconst = es.enter_context(tc.tile_pool(name="gconst", bufs=1))
        identb = gconst.tile([128, 128], BF16, name="identb")
        make_identity(nc, identb[:])
        identf = gconst.tile([128, 128], F32, name="identf")
        make_identity(nc, identf[:])
        eps_t = gconst.tile([128, 1], F32)
        nc.vector.memset(eps_t[:], EPS)
        onesb = gconst.tile([1, 128], BF16, name="onesb")
        nc.vector.memset(onesb[:], 1.0)
        bcp_row = gconst.tile([1, c.D], BF16, name="bcp_row")
        nc.sync.dma_start(out=bcp_row[:], in_=bcp_in[:, :])
        bmp_row = gconst.tile([1, c.D], BF16, name="bmp_row")
        nc.sync.dma_start(out=bmp_row[:], in_=bmp_in[:, :])
        battn_qk = gconst.tile([128, 2 * c.DC], F32, name="battn")
        nc.sync.dma_start(out=battn_qk[:], in_=battn_qk_in[:, :])
        bfc = gconst.tile([128, c.GB], F32, name="bfc")
        nc.sync.dma_start(out=bfc[:], in_=bfc_in[:, :])
        kofs = gconst.tile([128, c.KC], F32, name="kofs")
        nc.sync.dma_start(out=kofs[:], in_=kofs_in[:, :])
        masks = gconst.tile([128, c.KC, c.BS], BF16, name="masks")

        # ------------- persistent pools -------------
        # (pools reserve their footprint for their whole open lifetime, so
        # the c_proj/mproj/MLP pools open only after phase A)
        xlp = es.enter_context(tc.tile_pool(name="xloc", bufs=1, side="left"))
        kvqp = es.enter_context(tc.tile_pool(name="kvq", bufs=1,
                                             side="right"))

        xloc = []
        for tb in range(c.TB):
            t = xlp.tile([128, c.D], F32, tag=f"x{tb}", name=f"x{tb}")
            nc.sync.dma_start(out=t[:], in_=x_in[tb * 128:(tb + 1) * 128, :])
            xloc.append(t)

        ktp = kvqp.tile([128, c.DC, c.S], F8, name="ktp")
        vtt = kvqp.tile([128, 2, c.KCH, c.H, 65], F8, name="vtt")
        qtp = kvqp.tile([128, c.DC, c.T], F8, name="qtp")
        nc.vector.memset(vtt[:, :, :, :, 64:65], 1.0)

        # ================= phase A-head: LN1 + QKV-V =================
        es_ht = ExitStack()
        htp = es_ht.enter_context(tc.tile_pool(name="htp", bufs=1,
                                       side="right"))
        ht = htp.tile([128, c.DC, c.S], F8, name="ht")

        es_wa = ExitStack()
        wap = es_wa.enter_context(tc.tile_pool(name="wa", bufs=2,
                                       side="right"))
        wk = wap.tile([128, c.DC, c.D], F8, tag="wa", name="wk")
        nc.scalar.dma_start(
            out=wk[:],
            in_=w_attn[:, c.D:2 * c.D].rearrange("(i p) f -> p i f", p=128))
        wq = wap.tile([128, c.DC, c.D], F8, tag="wa", name="wq")
        nc.scalar.dma_start(
            out=wq[:],
            in_=w_attn[:, 0:c.D].rearrange("(i p) f -> p i f", p=128))

        NI = c.DC // 2  # DoubleRow contraction steps over D

        with (
            tc.tile_pool(name="xs", bufs=4) as xsp,
            tc.tile_pool(name="lnp", bufs=4) as lnp,
            tc.tile_pool(name="wvp", bufs=1, side="right") as wvp,
            tc.tile_pool(name="ps_tr", bufs=2, space="PSUM") as ps_tr,
            tc.tile_pool(name="ps_v", bufs=2, space="PSUM") as psv,
        ):
            wv = wvp.tile([128, c.DC, c.D], F8, name="wv")
            nc.scalar.dma_start(
                out=wv[:],
                in_=w_attn[:, 2 * c.D:3 * c.D].rearrange(
                    "(i p) f -> p i f", p=128))
            def v_chunk(g):
                for tb in range(4 * g, 4 * g + 4):
                    tbs = slice(tb * 128, (tb + 1) * 128)
                    pss = [psv.tile([128, 512], F32, tag="ps", name=f"psv{q}")
                           for q in range(2)]
                    for i in range(NI):
                        for vh in range(2):
                            nc.tensor.matmul(
                                pss[vh][:], ht[:, 2 * i:2 * i + 2, tbs],
                                wv[:, 2 * i:2 * i + 2,
                                   vh * 512:(vh + 1) * 512],
                                start=(i == 0), stop=(i == NI - 1),
                                perf_mode=DR)
                    for vh in range(2):
                        dst = vtt[:, tb // c.KCH, tb % c.KCH,
                                  vh * 8:(vh + 1) * 8, 0:64]
                        nc.scalar.activation(dst, pss[vh][:], AF.Identity,
                                             scale=inv_w)

            for tb in range(c.KT):
                if tb < c.TB:
                    src = xloc[tb]
                else:
                    src = xsp.tile([128, c.D], F32, tag="xs")
                    nc.sync.dma_start(
                        out=src[:], in_=x_in[tb * 128:(tb + 1) * 128, :])
                st = lnp.tile([128, 2, 6], F32, tag="ast")
                for sg in range(2):
                    nc.vector.bn_stats(
                        out=st[:, sg, :], in_=src[:, sg * 512:(sg + 1) * 512])
                mv = lnp.tile([128, 2], F32, tag="amv")
                nc.vector.bn_aggr(out=mv[:], in_=st[:])
                sd = lnp.tile([128, 1], F32, tag="asd")
                nc.scalar.activation(sd[:], mv[:, 1:2], AF.Sqrt,
                                     bias=eps_t[:, 0:1])
                rs = lnp.tile([128, 1], F32, tag="ars")
                nc.vector.reciprocal(rs[:], sd[:])
                nrm = lnp.tile([128, c.D], BF16, tag="an")
                nc.vector.tensor_scalar(
                    out=nrm[:], in0=src[:], scalar1=mv[:, 0:1],
                    scalar2=rs[:, 0:1], op0=ALU.subtract, op1=ALU.mult)
                for i2 in range(2):
                    pt = ps_tr.tile([128, 512], BF16, tag="atr")
                    for j in range(4):
                        ch = 4 * i2 + j
                        nc.tensor.matmul(
                            pt[:, j * 128:(j + 1) * 128],
                            nrm[:, ch * 128:(ch + 1) * 128], identb[:],
                            is_transpose=True, start=(j == 0), stop=(j == 3))
                    dst = ht[:, 4 * i2:4 * i2 + 4, tb * 128:(tb + 1) * 128]
                    if (tb + i2) % 2 == 0:
                        nc.vector.tensor_copy(dst, pt[:])
                    else:
                        nc.scalar.activation(dst, pt[:], AF.Identity)
                if tb % 4 == 3:
                    v_chunk(tb // 4)

        # late-opening persistent pools (phase A needed the headroom)
        wcp = es.enter_context(tc.tile_pool(name="wc", bufs=1, side="left"))
        atp = es.enter_context(tc.tile_pool(name="atp", bufs=1, side="left"))
        mgp = es.enter_context(tc.tile_pool(name="mgp", bufs=1, side="left"))
        wmp = es.enter_context(tc.tile_pool(name="wm", bufs=1, side="left"))

        # c_proj weights prefetch on the sync queue; mproj slabs stream
        # per (half, fh) later (SBUF is too tight to keep them resident)
        wc = wcp.tile([128, c.DC, c.D], F8, name="wc")
        nc.sync.dma_start(
            out=wc[:], in_=w_cproj[:, :].rearrange("(i p) f -> p i f", p=128))

        def load_mslab(fh):
            wm = wmp.tile([128, c.GB, 512], F8, tag="wm", name=f"wm{fh}")
            nc.sync.dma_start(
                out=wm[:],
                in_=w_mproj[:, fh * 512:(fh + 1) * 512].rearrange(
                    "(g p) f -> p g f", p=128))
            rm = wmp.tile([128, c.GB, 512], F8, tag="rm", name=f"rm{fh}")
            nc.sync.dma_start(
                out=rm[:],
                in_=r_mproj[:, fh * 512:(fh + 1) * 512].rearrange(
                    "(g p) f -> p g f", p=128))
            return wm, rm

        # mask bias matrices (GPSIMD; SBUF only) from a short-lived qidx tile
        with tc.tile_pool(name="qip", bufs=1) as qip:
            qidx = qip.tile([128, c.T], F32, name="qidx")
            nc.sync.dma_start(out=qidx[:], in_=bcast(qidx_in))
            for kc in range(c.KC):
                s_min = (kc % c.KCH) // c.CPB
                qsl = slice(s_min * c.BS, (s_min + 1) * c.BS)
                nc.gpsimd.tensor_scalar(
                    out=masks[:, kc, :], in0=qidx[:, qsl],
                    scalar1=kofs[:, kc:kc + 1], scalar2=NEGB,
                    op0=ALU.is_lt, op1=ALU.mult)

        # per-half reused activity tiles (allocated only now — phase A
        # needs the SBUF headroom)
        at2 = [atp.tile([128, c.DC, 512], F8, name=f"at{i}")
               for i in range(2)]
        mt = mgp.tile([128, c.DC, 512], F8, name="mt")
        gt = mgp.tile([128, c.GB, 512], F8, name="gt")

        # ============ interleaved region: K/Q + attention + MLP ============
        with (
            tc.tile_pool(name="pt", bufs=6) as ptp,
            tc.tile_pool(name="rec", bufs=2) as recp,
            tc.tile_pool(name="lnp2", bufs=2) as lnp2,
            tc.tile_pool(name="yout", bufs=2) as yop,
            tc.tile_pool(name="ps_qk", bufs=2, space="PSUM") as psqk,
            tc.tile_pool(name="ps_o", bufs=1, space="PSUM") as pso,
            tc.tile_pool(name="misc", bufs=2, space="PSUM") as miscp,
        ):
            def mtile():
                return miscp.tile([128, 512], F32, tag="m", name="m")

            # ---- filler pieces ----
            def kq_piece(m):
                def emit():
                    msl = slice(m * 128, (m + 1) * 128)
                    for th in range(4):
                        ps = mtile()
                        for i in range(NI):
                            nc.tensor.matmul(
                                ps[:], wk[:, 2 * i:2 * i + 2, msl],
                                ht[:, 2 * i:2 * i + 2,
                                   th * 512:(th + 1) * 512],
                                start=(i == 0), stop=(i == NI - 1),
                                perf_mode=DR)
                        nc.vector.tensor_scalar(
                            out=ktp[:, m, th * 512:(th + 1) * 512],
                            in0=ps[:], scalar1=1.0 / (WS / KQS),
                            scalar2=battn_qk[:, c.DC + m:c.DC + m + 1],
                            op0=ALU.mult, op1=ALU.add)
                    for th in range(2):
                        ps = mtile()
                        for i in range(NI):
                            nc.tensor.matmul(
                                ps[:], wq[:, 2 * i:2 * i + 2, msl],
                                ht[:, 2 * i:2 * i + 2,
                                   th * 512:(th + 1) * 512],
                                start=(i == 0), stop=(i == NI - 1),
                                perf_mode=DR)
                        nc.vector.tensor_scalar(
                            out=qtp[:, m, th * 512:(th + 1) * 512],
                            in0=ps[:], scalar1=1.0 / (WS / KQS),
                            scalar2=battn_qk[:, m:m + 1],
                            op0=ALU.mult, op1=ALU.add)
                return emit

            stats_box = {}

            def cproj_piece(tb, gi):
                """c_proj + residual for one tile, plus LN2 bn-stats.  The
                sqrt lives in ln2_piece so the 4 tiles share ONE ACT sqrt
                (each Exp<->Sqrt table switch costs 1283ns)."""
                def emit():
                    tbl = slice((tb - 4 * gi) * 128, (tb - 4 * gi + 1) * 128)
                    NC = c.DC // 2
                    for fh in range(2):
                        fsl = slice(fh * 512, (fh + 1) * 512)
                        ps = mtile()
                        for i in range(NC):
                            nc.tensor.matmul(
                                ps[:], at2[gi][:, 2 * i:2 * i + 2, tbl],
                                wc[:, 2 * i:2 * i + 2, fsl],
                                start=(i == 0), stop=False, perf_mode=DR)
                        nc.tensor.matmul(ps[:], onesb[:], bcp_row[0:1, fsl],
                                         start=False, stop=True)
                        # x2 written in place into the residual trunk
                        nc.vector.tensor_add(xloc[tb][:, fsl], ps[:],
                                             xloc[tb][:, fsl])
                    if tb % 4 == 0:
                        stats_box["mv"] = lnp2.tile([128, 4, 2], F32,
                                                    tag="dmv", name="dmv")
                    src = xloc[tb]
                    st = lnp2.tile([128, 2, 6], F32, tag="dst")
                    for sg in range(2):
                        nc.vector.bn_stats(
                            out=st[:, sg, :],
                            in_=src[:, sg * 512:(sg + 1) * 512])
                    nc.vector.bn_aggr(out=stats_box["mv"][:, tb % 4, :],
                                      in_=st[:])
                return emit

            def ln2_piece(gi):
                """one batched sqrt for the half's 4 tiles, then normalize +
                transpose + evict each tile into mt."""
                def emit():
                    mv = stats_box["mv"]
                    sd = lnp2.tile([128, 4], F32, tag="dsd")
                    nc.scalar.activation(sd[:], mv[:, :, 1], AF.Sqrt,
                                         bias=eps_t[:, 0:1])
                    rs = lnp2.tile([128, 4], F32, tag="drs")
                    nc.vector.reciprocal(rs[:], sd[:])
                    for tb in range(4 * gi, 4 * gi + 4):
                        tbl = slice((tb - 4 * gi) * 128,
                                    (tb - 4 * gi + 1) * 128)
                        i4 = tb % 4
                        nrm = lnp2.tile([128, c.D], F32, tag="dn")
                        nc.vector.tensor_scalar(
                            out=nrm[:], in0=xloc[tb][:],
                            scalar1=mv[:, i4, 0:1], scalar2=rs[:, i4:i4 + 1],
                            op0=ALU.subtract, op1=ALU.mult)
                        for i2 in range(2):
                            pt = mtile()
                            for j in range(4):
                                ch = 4 * i2 + j
                                nc.tensor.matmul(
                                    pt[:, j * 128:(j + 1) * 128],
                                    nrm[:, ch * 128:(ch + 1) * 128],
                                    identf[:],
                                    is_transpose=True, start=(j == 0),
                                    stop=(j == 3))
                            nc.vector.tensor_copy(
                                mt[:, 4 * i2:4 * i2 + 4, tbl], pt[:])
                return emit

            wfp_box = {}
            es_fc = ExitStack()

            def fc_piece(gb0, gi):
                # 16 hidden-blocks per piece: the gelus cluster so the
                # Exp<->Gelu ACT table switch is paid twice per piece, not
                # per 4-block group
                def emit():
                    if "p" not in wfp_box:
                        wfp_box["p"] = es_fc.enter_context(
                            tc.tile_pool(name="wf", bufs=2, side="left"))
                    wfp = wfp_box["p"]
                    for gb in range(gb0, gb0 + 16):
                        if gb % 4 == 0:
                            j = gb // 4
                            wf = wfp.tile([128, c.DC, 512], F8, tag="wf",
                                          name=f"wf{gi}_{gb}")
                            nc.scalar.dma_start(
                                out=wf[:],
                                in_=w_fc[:, j * 512:(j + 1) * 512].rearrange(
                                    "(i p) f -> p i f", p=128))
                            rf = wfp.tile([128, c.DC, 512], F8, tag="rf",
                                          name=f"rf{gi}_{gb}")
                            nc.scalar.dma_start(
                                out=rf[:],
                                in_=r_fc[:, j * 512:(j + 1) * 512].rearrange(
                                    "(i p) f -> p i f", p=128))
                            emit.wf, emit.rf = wf, rf
                        wf, rf = emit.wf, emit.rf
                        gl = (gb % 4) * 128
                        ps = mtile()
                        for wslab, first in ((wf, True), (rf, False)):
                            for i in range(NI):
                                nc.tensor.matmul(
                                    ps[:],
                                    wslab[:, 2 * i:2 * i + 2, gl:gl + 128],
                                    mt[:, 2 * i:2 * i + 2, :],
                                    start=(first and i == 0),
                                    stop=(not first and i == NI - 1),
                                    perf_mode=DR)
                        nc.scalar.activation(
                            gt[:, gb, :], ps[:], AF.Gelu_apprx_tanh,
                            bias=bfc[:, gb:gb + 1], scale=inv_w)
                return emit

            yo_tiles = {}
            mslab = {}

            def mslab_prefetch(fh):
                # issue the slab DMA well before the first mproj piece needs
                # it (the DMA still waits on the previous slab's last reader)
                def emit():
                    mslab[fh] = load_mslab(fh)
                return emit

            def mproj_piece(tb, gi, fh):
                def emit():
                    tbl = slice((tb - 4 * gi) * 128, (tb - 4 * gi + 1) * 128)
                    NG = c.GB // 2
                    wm, rm = mslab[fh]
                    yo = yop.tile([128, 512], F32, tag="yo",
                                  name=f"yo{tb}_{fh}")
                    fsl = slice(fh * 512, (fh + 1) * 512)
                    ps = mtile()
                    for wslab, first in ((wm, True), (rm, False)):
                        for g in range(NG):
                            nc.tensor.matmul(
                                ps[:], gt[:, 2 * g:2 * g + 2, tbl],
                                wslab[:, 2 * g:2 * g + 2, :],
                                start=(first and g == 0),
                                stop=False, perf_mode=DR)
                    nc.tensor.matmul(ps[:], onesb[:], bmp_row[0:1, fsl],
                                     start=False, stop=True)
                    nc.vector.tensor_add(yo[:], ps[:], xloc[tb][:, fsl])
                    nc.sync.dma_start(
                        out=y_out[tb * 128:(tb + 1) * 128, fsl], in_=yo[:])
                return emit

            # ---- attention for one (jj, gi) block ----
            groups = [list(range(c.SPG * gi, c.SPG * (gi + 1)))
                      for gi in range(c.SLOTS // c.SPG)]

            def attention(jj, gi):
                # QK/exp run LAG plain-steps ahead of AV so the AV of the
                # next block never stalls the in-order PE queue on the
                # previous block's softmax-normalize reads of `pos`.
                LAG = 2
                g = groups[gi]
                s0, s3 = g[0], g[-1]
                n_loc = (s3 + 1) * c.CPB
                pos = pso.tile([65, 2, 512], F32, tag="po", name="po")
                pending = deque()

                def emit_av(item):
                    loc, hp, h, lo, pt, w = item
                    nc.tensor.matmul(
                        pos[:, hp, (lo - s0) * c.BS:512],
                        vtt[:, :, loc, h, :], pt[:, :, 0:w],
                        start=(loc == 0), stop=(loc == n_loc - 1),
                        perf_mode=DR)

                for loc in range(n_loc):
                    lo = max(s0, loc // c.CPB)
                    w = (s3 - lo + 1) * c.BS
                    qsl = slice(lo * c.BS, (s3 + 1) * c.BS)
                    diag = loc // c.CPB >= s0
                    for hp in range(c.HPB):
                        h = c.HPB * jj + hp
                        base = hp * 64
                        ps2 = psqk.tile([128, 2, 512], F32, tag="qk")
                        pt = ptp.tile([128, 2, 512], F8, tag="pt")
                        for ix in range(2):
                            kc = loc + ix * c.KCH
                            nc.tensor.matmul(
                                ps2[:, ix, 0:w],
                                ktp[base:base + 64, jj,
                                    kc * 128:(kc + 1) * 128],
                                qtp[base:base + 64, jj, qsl],
                                start=True, stop=not diag)
                            if diag:
                                nc.tensor.matmul(
                                    ps2[:, ix, 0:c.BS], identb[:],
                                    masks[:, kc, :],
                                    start=False, stop=True)
                        nc.scalar.activation(pt[:, :, 0:w], ps2[:, :, 0:w],
                                             AF.Exp, scale=1.0 / (KQS * KQS))
                        pending.append((loc, hp, h, lo, pt, w))
                        while len(pending) > LAG * c.HPB:
                            emit_av(pending.popleft())
                while pending:
                    emit_av(pending.popleft())
                for hp in range(c.HPB):
                    base = hp * 64
                    rec = recp.tile([1, 512], F32, tag="rec")
                    nc.vector.reciprocal(rec[:], pos[64:65, hp, :])
                    bcs = recp.tile([64, 512], F32, tag="bcs")
                    nc.gpsimd.partition_broadcast(bcs[:], rec[:])
                    nc.vector.tensor_tensor(
                        out=at2[gi][base:base + 64, jj, :],
                        in0=pos[0:64, hp, :], in1=bcs[:], op=ALU.mult)

            # ---- the interleaved emission ----
            filler = deque(kq_piece(m) for m in range(c.DC))
            for gi in range(2):
                for jj in range(c.DC):
                    # keep the K/Q chunk needed by the NEXT jj ahead of
                    # its attention block during gi=0
                    while (gi == 0 and filler
                           and len(filler) > (c.DC - 2 - jj)):
                        filler.popleft()()
                    attention(jj, gi)
                    if gi == 1:
                        for _ in range(3):
                            if filler:
                                filler.popleft()()
                while gi == 0 and filler:
                    filler.popleft()()
                if gi == 0:
                    es_wa.close()
                    es_ht.close()
                # queue this half's MLP chain as filler for the next half.
                # mproj fh-order alternates per half so the still-resident
                # slab pair is reused (3 slab loads total instead of 4)
                for tb in range(4 * gi, 4 * gi + 4):
                    filler.append(cproj_piece(tb, gi))
                filler.append(ln2_piece(gi))
                fhs = (0, 1) if gi == 0 else (1, 0)
                filler.append(mslab_prefetch(fhs[0]) if gi == 0
                              else fc_piece(0, gi))
                filler.append(fc_piece(0, gi) if gi == 0
                              else fc_piece(16, gi))
                if gi == 0:
                    filler.append(fc_piece(16, gi))
                for tb in range(4 * gi, 4 * gi + 4):
                    filler.append(mproj_piece(tb, gi, fhs[0]))
                filler.append(mslab_prefetch(fhs[1]))
                for tb in range(4 * gi, 4 * gi + 4):
                    filler.append(mproj_piece(tb, gi, fhs[1]))
            while filler:
                filler.popleft()()
            es_fc.close()

    nc.compile()
    return nc


def core_rows(cfg, half):
    """absolute sequence rows owned by a core with parity half"""
    c = cfg
    loc = np.arange(c.T)
    return (2 * (loc // c.BS) + half) * c.BS + loc % c.BS


def make_core_inputs(cfg: Cfg, x, ln1_w, ln1_b, W_attn, b_attn, W_cproj,
                     b_cproj, ln2_w, ln2_b, W_fc, b_fc, W_mproj, b_mproj):
    """Split full inputs into one in_map per core."""
    c = cfg
    f32 = np.float32
    f8 = ml_dtypes.float8_e4m3fn
    bf = ml_dtypes.bfloat16

    def q8w(w):
        return np.ascontiguousarray(np.asarray(w, f32) * WS).astype(f8)

    # fold LN1 affine + query scale into W_attn / b_attn
    ln1_w = np.asarray(ln1_w, f32)
    ln1_b = np.asarray(ln1_b, f32)
    Wa = np.asarray(W_attn, f32) * ln1_w[:, None]
    ba = np.asarray(b_attn, f32) + ln1_b @ np.asarray(W_attn, f32)
    qs = 1.0 / math.sqrt(c.HD)
    Wa = Wa.copy()
    Wa[:, :c.D] *= qs
    ba = ba.copy()
    ba[:c.D] *= qs

    # fold LN2 affine into W_fc / b_fc
    ln2_w = np.asarray(ln2_w, f32)
    ln2_b = np.asarray(ln2_b, f32)
    Wf = np.asarray(W_fc, f32) * ln2_w[:, None]
    bf_fold = np.asarray(b_fc, f32) + ln2_b @ np.asarray(W_fc, f32)

    # fold the V bias through the softmax (weights sum to 1): it becomes a
    # constant shift of the attention output -> bv @ W_cproj joins b_cproj
    bv = ba[2 * c.D:3 * c.D]
    bcp_tot = np.asarray(b_cproj, f32) + bv @ np.asarray(W_cproj, f32)

    Wf8 = q8w(Wf)
    Rf8 = np.ascontiguousarray(
        np.asarray(Wf, f32) * WS - np.asarray(Wf8, f32)).astype(f8)
    Wm8 = q8w(W_mproj)
    Rm8 = np.ascontiguousarray(
        np.asarray(W_mproj, f32) * WS - np.asarray(Wm8, f32)).astype(f8)

    shared = {
        "w_attn": q8w(Wa),
        "w_cproj": q8w(W_cproj),
        "w_fc": Wf8,
        "r_fc": Rf8,
        "w_mproj": Wm8,
        "r_mproj": Rm8,
        "bcp": np.ascontiguousarray(
            TRUNK * bcp_tot).astype(bf).reshape(1, c.D),
        "bmp": np.ascontiguousarray(
            TRUNK * np.asarray(b_mproj, f32)).astype(bf).reshape(1, c.D),
        "bfc": np.ascontiguousarray(bf_fold.reshape(c.GB, 128).T),
        # k/q evictions: out = psum/(WS/KQS) + KQS*bias
        "battn_qk": np.ascontiguousarray(
            KQS * ba[:2 * c.D].reshape(2 * c.DC, 128).T),
    }

    x = np.asarray(x, f32)
    in_maps = []
    for core in range(c.n_cores):
        b, half = core // 2, core % 2
        own = core_rows(c, half)
        peer = core_rows(c, 1 - half)
        perm = np.concatenate([own, peer])
        m = dict(shared)
        m["x"] = np.ascontiguousarray(TRUNK * x[b][perm])
        m["qidx"] = own.astype(f32).reshape(1, c.T)
        kofs = np.empty((128, c.KC), f32)
        for kc in range(c.KC):
            kofs[:, kc] = perm[kc * 128 + np.arange(128)]
        m["kofs"] = kofs
        in_maps.append(m)
    return in_maps


_NC_CACHE = {}


def get_nc(cfg: Cfg):
    key = (cfg.B, cfg.S, cfg.D, cfg.H, cfg.F, cfg.BS)
    if key not in _NC_CACHE:
        _NC_CACHE[key] = build(cfg)
    return _NC_CACHE[key]


def kernel(**inputs) -> np.ndarray:
    from concourse.bass_utils import run_bass_kernel_spmd

    cfg = Cfg()
    nc = get_nc(cfg)
    in_maps = make_core_inputs(cfg, **inputs)
    res = run_bass_kernel_spmd(nc, in_maps, core_ids=list(range(cfg.n_cores)))
    B, S, D = cfg.B, cfg.S, cfg.D
    out = np.empty((B, S, D), np.float32)
    inv_trunk = 1.0 / TRUNK
    for core in range(cfg.n_cores):
        b, half = core // 2, core % 2
        out[b, core_rows(cfg, half), :] = inv_trunk * res.results[core]["y"]
    return out


# revision 31
# speedup vs baseline: 1.0281x; 1.0169x over previous
"""Single transformer block on 8 NeuronCores — collective-free, v3.

Sharding: core c = (batch b=c//2, parity p=c%2). Each core receives the FULL
sequence of its batch, permuted to [own-stripe | peer-stripe] order, and
recomputes K and V for all 2048 tokens locally.  Q / attention / c_proj / MLP
cover only the core's 1024 own (striped) tokens.

v3 structure (driven by the TimelineSim cost model):
  - exp() can only run on the ACT engine and is the hard floor of the
    attention phase (~150us); the MLP is the PE floor (~110us).  v3 runs the
    two query-groups (512 own tokens each) OUTER, so after group 0 finishes
    attention, its c_proj/LN2/fc/mproj work is emitted interleaved with
    group 1's attention — the PE-heavy MLP fills the PE slack under group
    1's ACT-bound exp stream.  K^T/Q^T projection passes fill group 0.
  - The residual trunk is carried SCALED BY 64 on-chip (xloc = 64*x host
    side; host divides the output by 64).  Weights are fp8 at x64,
    activations fp8 at true scale (k/q at x8), so every PSUM arrives at
    64x (scores: 64x) truth and all rescales fold into activation-scale
    slots that are free.  x2 is written in place into xloc.
  - fc and mproj run fp8 DoubleRow with a WEIGHT-RESIDUAL second pass
    (W ~= Q(64W) + Q(64W - Q(64W))): half the bf16 cost, and the dominant
    weight-quantization error term vanishes.  c_proj is plain fp8 DR.
  - All biases are free: LN affines fold into following matmuls host-side;
    bv folds into bcp (softmax weights sum to 1); bcp/bmp ride into the
    c_proj/mproj PSUM as rank-1 matmuls; bk/bq fold into K/Q evictions;
    bfc into the gelu activation.
  - Causal masking: a -1920 (= 64 * -30) additive bias matrix accumulated
    onto the diagonal score block by an identity matmul; exp flushes those
    lanes to 0 in fp8.  No DVE mask multiplies.
  - Softmax denominators: V carries a ones column so AV row 64 is the
    denominator; reciprocal + P*(1/d) on DVE, partition-broadcast on the
    (PSUM-blind but otherwise idle) GPSIMD engine.
  - PSUM budget (8 banks): QK 2x[128,2,512] (4) + AV [65,2,512] (2) +
    a single shared [128,512] f32 scratch tag (2) used by every filler
    stage: k/q passes, c_proj, LN2 transposes (f32), fc, mproj.
"""

import math
from collections import deque
from contextlib import ExitStack

import numpy as np
import ml_dtypes

import concourse.bacc as bacc
import concourse.bass as bass
import concourse.mybir as mybir
import concourse.tile as tile
from concourse.masks import make_identity

F32 = mybir.dt.float32
BF16 = mybir.dt.bfloat16
F8 = mybir.dt.float8e4
AF = mybir.ActivationFunctionType
ALU = mybir.AluOpType
DR = mybir.MatmulPerfMode.DoubleRow

EPS = 1e-5
TRUNK = 64.0       # residual trunk scale carried on-chip
WS = 64.0          # fp8 weight scale
KQS = 8.0          # k/q fp8 storage scale (scores come out at 64x)
NEGB = -30.0 * 64  # additive mask bias at score-PSUM scale


class Cfg:
    def __init__(self, B=4, S=2048, D=1024, H=16, F=4096, n_cores=8, BS=128):
        self.B, self.S, self.D, self.H, self.F = B, S, D, H, F
        self.n_cores = n_cores
        assert n_cores == 2 * B
        self.HD = D // H
        assert self.HD == 64
        self.T = S // 2            # tokens owned per core
        self.KT = S // 128         # token 128-tiles, full sequence
        self.TB = self.T // 128    # token 128-tiles, local
        self.DC = D // 128         # contraction chunks over D
        self.KC = S // 128         # key 128-chunks over full sequence
        self.GB = F // 128         # MLP hidden 128-blocks
        self.HPB = 128 // self.HD  # heads per 128-feature block (=2)
        self.BS = BS               # stripe block (q-slot) size
        self.SLOTS = self.T // self.BS
        self.SPG = 512 // self.BS  # slots per 512-wide attention group
        self.KCH = self.KC // 2    # chunks per parity half
        self.CPB = self.BS // 128  # key chunks per stripe block


def build(cfg: Cfg):
    c = cfg
    nc = bacc.Bacc(None, target_bir_lowering=False)
    inv_w = 1.0 / WS

    # ---------------- I/O ----------------
    x_in = nc.dram_tensor("x", [c.S, c.D], F32, kind="ExternalInput")
    w_attn = nc.dram_tensor("w_attn", [c.D, 3 * c.D], F8, kind="ExternalInput")
    w_cproj = nc.dram_tensor("w_cproj", [c.D, c.D], F8, kind="ExternalInput")
    w_fc = nc.dram_tensor("w_fc", [c.D, c.F], F8, kind="ExternalInput")
    r_fc = nc.dram_tensor("r_fc", [c.D, c.F], F8, kind="ExternalInput")
    w_mproj = nc.dram_tensor("w_mproj", [c.F, c.D], F8, kind="ExternalInput")
    r_mproj = nc.dram_tensor("r_mproj", [c.F, c.D], F8, kind="ExternalInput")
    battn_qk_in = nc.dram_tensor("battn_qk", [128, 2 * c.DC], F32,
                                 kind="ExternalInput")
    bcp_in = nc.dram_tensor("bcp", [1, c.D], BF16, kind="ExternalInput")
    bmp_in = nc.dram_tensor("bmp", [1, c.D], BF16, kind="ExternalInput")
    bfc_in = nc.dram_tensor("bfc", [128, c.GB], F32, kind="ExternalInput")
    qidx_in = nc.dram_tensor("qidx", [1, c.T], F32, kind="ExternalInput")
    kofs_in = nc.dram_tensor("kofs", [128, c.KC], F32, kind="ExternalInput")
    y_out = nc.dram_tensor("y", [c.T, c.D], F32, kind="ExternalOutput")

    def bcast(dram, p=128):
        return bass.AP(tensor=dram, offset=0, ap=[[0, p], [1, dram.shape[1]]])

    with tile.TileContext(nc) as tc, ExitStack() as es:
        gconst = es.enter_context(tc.tile_pool(name="gconst", bufs=1))
        identb = gconst.tile([128, 128], BF16, name="identb")
        make_identity(nc, identb[:])
        identf = gconst.tile([128, 128], F32, name="identf")
        make_identity(nc, identf[:])
        eps_t = gconst.tile([128, 1], F32)
        nc.vector.memset(eps_t[:], EPS)
        onesb = gconst.tile([1, 128], BF16, name="onesb")
        nc.vector.memset(onesb[:], 1.0)
        bcp_row = gconst.tile([1, c.D], BF16, name="bcp_row")
        nc.sync.dma_start(out=bcp_row[:], in_=bcp_in[:, :])
        bmp_row = gconst.tile([1, c.D], BF16, name="bmp_row")
        nc.sync.dma_start(out=bmp_row[:], in_=bmp_in[:, :])
        battn_qk = gconst.tile([128, 2 * c.DC], F32, name="battn")
        nc.sync.dma_start(out=battn_qk[:], in_=battn_qk_in[:, :])
        bfc = gconst.tile([128, c.GB], F32, name="bfc")
        nc.sync.dma_start(out=bfc[:], in_=bfc_in[:, :])
        kofs = gconst.tile([128, c.KC], F32, name="kofs")
        nc.sync.dma_start(out=kofs[:], in_=kofs_in[:, :])
        masks = gconst.tile([128, c.KC, c.BS], BF16, name="masks")

        # ------------- persistent pools -------------
        # (pools reserve their footprint for their whole open lifetime, so
        # the c_proj/mproj/MLP pools open only after phase A)
        xlp = es.enter_context(tc.tile_pool(name="xloc", bufs=1, side="left"))
        kvqp = es.enter_context(tc.tile_pool(name="kvq", bufs=1,
                                             side="right"))

        xloc = []
        for tb in range(c.TB):
            t = xlp.tile([128, c.D], F32, tag=f"x{tb}", name=f"x{tb}")
            nc.sync.dma_start(out=t[:], in_=x_in[tb * 128:(tb + 1) * 128, :])
            xloc.append(t)

        ktp = kvqp.tile([128, c.DC, c.S], F8, name="ktp")
        vtt = kvqp.tile([128, 2, c.KCH, c.H, 65], F8, name="vtt")
        qtp = kvqp.tile([128, c.DC, c.T], F8, name="qtp")
        nc.vector.memset(vtt[:, :, :, :, 64:65], 1.0)

        # ================= phase A-head: LN1 + QKV-V =================
        es_ht = ExitStack()
        htp = es_ht.enter_context(tc.tile_pool(name="htp", bufs=1,
                                       side="right"))
        ht = htp.tile([128, c.DC, c.S], F8, name="ht")

        es_wa = ExitStack()
        wap = es_wa.enter_context(tc.tile_pool(name="wa", bufs=2,
                                       side="right"))
        wk = wap.tile([128, c.DC, c.D], F8, tag="wa", name="wk")
        nc.scalar.dma_start(
            out=wk[:],
            in_=w_attn[:, c.D:2 * c.D].rearrange("(i p) f -> p i f", p=128))
        wq = wap.tile([128, c.DC, c.D], F8, tag="wa", name="wq")
        nc.scalar.dma_start(
            out=wq[:],
            in_=w_attn[:, 0:c.D].rearrange("(i p) f -> p i f", p=128))

        NI = c.DC // 2  # DoubleRow contraction steps over D

        with (
            tc.tile_pool(name="xs", bufs=4) as xsp,
            tc.tile_pool(name="lnp", bufs=4) as lnp,
            tc.tile_pool(name="wvp", bufs=1, side="right") as wvp,
            tc.tile_pool(name="ps_tr", bufs=2, space="PSUM") as ps_tr,
            tc.tile_pool(name="ps_v", bufs=2, space="PSUM") as psv,
        ):
            wv = wvp.tile([128, c.DC, c.D], F8, name="wv")
            nc.scalar.dma_start(
                out=wv[:],
                in_=w_attn[:, 2 * c.D:3 * c.D].rearrange(
                    "(i p) f -> p i f", p=128))
            def v_chunk(g):
                for tb in range(4 * g, 4 * g + 4):
                    tbs = slice(tb * 128, (tb + 1) * 128)
                    pss = [psv.tile([128, 512], F32, tag="ps", name=f"psv{q}")
                           for q in range(2)]
                    for i in range(NI):
                        for vh in range(2):
                            nc.tensor.matmul(
                                pss[vh][:], ht[:, 2 * i:2 * i + 2, tbs],
                                wv[:, 2 * i:2 * i + 2,
                                   vh * 512:(vh + 1) * 512],
                                start=(i == 0), stop=(i == NI - 1),
                                perf_mode=DR)
                    for vh in range(2):
                        dst = vtt[:, tb // c.KCH, tb % c.KCH,
                                  vh * 8:(vh + 1) * 8, 0:64]
                        nc.scalar.activation(dst, pss[vh][:], AF.Identity,
                                             scale=inv_w)

            for tb in range(c.KT):
                if tb < c.TB:
                    src = xloc[tb]
                else:
                    src = xsp.tile([128, c.D], F32, tag="xs")
                    nc.sync.dma_start(
                        out=src[:], in_=x_in[tb * 128:(tb + 1) * 128, :])
                st = lnp.tile([128, 2, 6], F32, tag="ast")
                for sg in range(2):
                    nc.vector.bn_stats(
                        out=st[:, sg, :], in_=src[:, sg * 512:(sg + 1) * 512])
                mv = lnp.tile([128, 2], F32, tag="amv")
                nc.vector.bn_aggr(out=mv[:], in_=st[:])
                sd = lnp.tile([128, 1], F32, tag="asd")
                nc.scalar.activation(sd[:], mv[:, 1:2], AF.Sqrt,
                                     bias=eps_t[:, 0:1])
                rs = lnp.tile([128, 1], F32, tag="ars")
                nc.vector.reciprocal(rs[:], sd[:])
                nrm = lnp.tile([128, c.D], BF16, tag="an")
                nc.vector.tensor_scalar(
                    out=nrm[:], in0=src[:], scalar1=mv[:, 0:1],
                    scalar2=rs[:, 0:1], op0=ALU.subtract, op1=ALU.mult)
                for i2 in range(2):
                    pt = ps_tr.tile([128, 512], BF16, tag="atr")
                    for j in range(4):
                        ch = 4 * i2 + j
                        nc.tensor.matmul(
                            pt[:, j * 128:(j + 1) * 128],
                            nrm[:, ch * 128:(ch + 1) * 128], identb[:],
                            is_transpose=True, start=(j == 0), stop=(j == 3))
                    dst = ht[:, 4 * i2:4 * i2 + 4, tb * 128:(tb + 1) * 128]
                    if (tb + i2) % 2 == 0:
                        nc.vector.tensor_copy(dst, pt[:])
                    else:
                        nc.scalar.activation(dst, pt[:], AF.Identity)
                if tb % 4 == 3:
                    v_chunk(tb // 4)

        # late-opening persistent pools (phase A needed the headroom)
        wcp = es.enter_context(tc.tile_pool(name="wc", bufs=1, side="left"))
        atp = es.enter_context(tc.tile_pool(name="atp", bufs=1, side="left"))
        mgp = es.enter_context(tc.tile_pool(name="mgp", bufs=1, side="left"))
        wmp = es.enter_context(tc.tile_pool(name="wm", bufs=1, side="left"))

        # c_proj weights prefetch on the sync queue; mproj slabs stream
        # per (half, fh) later (SBUF is too tight to keep them resident)
        wc = wcp.tile([128, c.DC, c.D], F8, name="wc")
        nc.sync.dma_start(
            out=wc[:], in_=w_cproj[:, :].rearrange("(i p) f -> p i f", p=128))

        def load_mslab(fh):
            wm = wmp.tile([128, c.GB, 512], F8, tag="wm", name=f"wm{fh}")
            nc.sync.dma_start(
                out=wm[:],
                in_=w_mproj[:, fh * 512:(fh + 1) * 512].rearrange(
                    "(g p) f -> p g f", p=128))
            rm = wmp.tile([128, c.GB, 512], F8, tag="rm", name=f"rm{fh}")
            nc.sync.dma_start(
                out=rm[:],
                in_=r_mproj[:, fh * 512:(fh + 1) * 512].rearrange(
                    "(g p) f -> p g f", p=128))
            return wm, rm

        # mask bias matrices (GPSIMD; SBUF only) from a short-lived qidx tile
        with tc.tile_pool(name="qip", bufs=1) as qip:
            qidx = qip.tile([128, c.T], F32, name="qidx")
            nc.sync.dma_start(out=qidx[:], in_=bcast(qidx_in))
            for kc in range(c.KC):
                s_min = (kc % c.KCH) // c.CPB
                qsl = slice(s_min * c.BS, (s_min + 1) * c.BS)
                nc.gpsimd.tensor_scalar(
                    out=masks[:, kc, :], in0=qidx[:, qsl],
                    scalar1=kofs[:, kc:kc + 1], scalar2=NEGB,
                    op0=ALU.is_lt, op1=ALU.mult)

        # per-half reused activity tiles (allocated only now — phase A
        # needs the SBUF headroom)
        at2 = [atp.tile([128, c.DC, 512], F8, name=f"at{i}")
               for i in range(2)]
        mt = mgp.tile([128, c.DC, 512], F8, name="mt")
        gt = mgp.tile([128, c.GB, 512], F8, name="gt")

        # ============ interleaved region: K/Q + attention + MLP ============
        with (
            tc.tile_pool(name="pt", bufs=6) as ptp,
            tc.tile_pool(name="rec", bufs=2) as recp,
            tc.tile_pool(name="lnp2", bufs=2) as lnp2,
            tc.tile_pool(name="yout", bufs=2) as yop,
            tc.tile_pool(name="ps_qk", bufs=2, space="PSUM") as psqk,
            tc.tile_pool(name="ps_o", bufs=1, space="PSUM") as pso,
            tc.tile_pool(name="misc", bufs=2, space="PSUM") as miscp,
        ):
            def mtile():
                return miscp.tile([128, 512], F32, tag="m", name="m")

            # ---- filler pieces ----
            def kq_piece(m):
                def emit():
                    msl = slice(m * 128, (m + 1) * 128)
                    for th in range(4):
                        ps = mtile()
                        for i in range(NI):
                            nc.tensor.matmul(
                                ps[:], wk[:, 2 * i:2 * i + 2, msl],
                                ht[:, 2 * i:2 * i + 2,
                                   th * 512:(th + 1) * 512],
                                start=(i == 0), stop=(i == NI - 1),
                                perf_mode=DR)
                        nc.vector.tensor_scalar(
                            out=ktp[:, m, th * 512:(th + 1) * 512],
                            in0=ps[:], scalar1=1.0 / (WS / KQS),
                            scalar2=battn_qk[:, c.DC + m:c.DC + m + 1],
                            op0=ALU.mult, op1=ALU.add)
                    for th in range(2):
                        ps = mtile()
                        for i in range(NI):
                            nc.tensor.matmul(
                                ps[:], wq[:, 2 * i:2 * i + 2, msl],
                                ht[:, 2 * i:2 * i + 2,
                                   th * 512:(th + 1) * 512],
                                start=(i == 0), stop=(i == NI - 1),
                                perf_mode=DR)
                        nc.vector.tensor_scalar(
                            out=qtp[:, m, th * 512:(th + 1) * 512],
                            in0=ps[:], scalar1=1.0 / (WS / KQS),
                            scalar2=battn_qk[:, m:m + 1],
                            op0=ALU.mult, op1=ALU.add)
                return emit

            stats_box = {}

            def cproj_piece(tb, gi):
                """c_proj + residual for one tile, plus LN2 bn-stats.  The
                sqrt lives in ln2_piece so the 4 tiles share ONE ACT sqrt
                (each Exp<->Sqrt table switch costs 1283ns)."""
                def emit():
                    tbl = slice((tb - 4 * gi) * 128, (tb - 4 * gi + 1) * 128)
                    NC = c.DC // 2
                    for fh in range(2):
                        fsl = slice(fh * 512, (fh + 1) * 512)
                        ps = mtile()
                        for i in range(NC):
                            nc.tensor.matmul(
                                ps[:], at2[gi][:, 2 * i:2 * i + 2, tbl],
                                wc[:, 2 * i:2 * i + 2, fsl],
                                start=(i == 0), stop=False, perf_mode=DR)
                        nc.tensor.matmul(ps[:], onesb[:], bcp_row[0:1, fsl],
                                         start=False, stop=True)
                        # x2 written in place into the residual trunk
                        nc.vector.tensor_add(xloc[tb][:, fsl], ps[:],
                                             xloc[tb][:, fsl])
                    if tb % 4 == 0:
                        stats_box["mv"] = lnp2.tile([128, 4, 2], F32,
                                                    tag="dmv", name="dmv")
                    src = xloc[tb]
                    st = lnp2.tile([128, 2, 6], F32, tag="dst")
                    for sg in range(2):
                        nc.vector.bn_stats(
                            out=st[:, sg, :],
                            in_=src[:, sg * 512:(sg + 1) * 512])
                    nc.vector.bn_aggr(out=stats_box["mv"][:, tb % 4, :],
                                      in_=st[:])
                return emit

            def ln2_piece(gi):
                """one batched sqrt for the half's 4 tiles, then normalize +
                transpose + evict each tile into mt."""
                def emit():
                    mv = stats_box["mv"]
                    sd = lnp2.tile([128, 4], F32, tag="dsd")
                    nc.scalar.activation(sd[:], mv[:, :, 1], AF.Sqrt,
                                         bias=eps_t[:, 0:1])
                    rs = lnp2.tile([128, 4], F32, tag="drs")
                    nc.vector.reciprocal(rs[:], sd[:])
                    for tb in range(4 * gi, 4 * gi + 4):
                        tbl = slice((tb - 4 * gi) * 128,
                                    (tb - 4 * gi + 1) * 128)
                        i4 = tb % 4
                        nrm = lnp2.tile([128, c.D], F32, tag="dn")
                        nc.vector.tensor_scalar(
                            out=nrm[:], in0=xloc[tb][:],
                            scalar1=mv[:, i4, 0:1], scalar2=rs[:, i4:i4 + 1],
                            op0=ALU.subtract, op1=ALU.mult)
                        for i2 in range(2):
                            pt = mtile()
                            for j in range(4):
                                ch = 4 * i2 + j
                                nc.tensor.matmul(
                                    pt[:, j * 128:(j + 1) * 128],
                                    nrm[:, ch * 128:(ch + 1) * 128],
                                    identf[:],
                                    is_transpose=True, start=(j == 0),
                                    stop=(j == 3))
                            nc.vector.tensor_copy(
                                mt[:, 4 * i2:4 * i2 + 4, tbl], pt[:])
                return emit

            wfp_box = {}
            es_fc = ExitStack()

            def fc_piece(gb0, gi):
                # 16 hidden-blocks per piece: the gelus cluster so the
                # Exp<->Gelu ACT table switch is paid twice per piece, not
                # per 4-block group
                def emit():
                    if "p" not in wfp_box:
                        wfp_box["p"] = es_fc.enter_context(
                            tc.tile_pool(name="wf", bufs=2, side="left"))
                    wfp = wfp_box["p"]
                    for gb in range(gb0, gb0 + 16):
                        if gb % 4 == 0:
                            j = gb // 4
                            wf = wfp.tile([128, c.DC, 512], F8, tag="wf",
                                          name=f"wf{gi}_{gb}")
                            nc.scalar.dma_start(
                                out=wf[:],
                                in_=w_fc[:, j * 512:(j + 1) * 512].rearrange(
                                    "(i p) f -> p i f", p=128))
                            rf = wfp.tile([128, c.DC, 512], F8, tag="rf",
                                          name=f"rf{gi}_{gb}")
                            nc.scalar.dma_start(
                                out=rf[:],
                                in_=r_fc[:, j * 512:(j + 1) * 512].rearrange(
                                    "(i p) f -> p i f", p=128))
                            emit.wf, emit.rf = wf, rf
                        wf, rf = emit.wf, emit.rf
                        gl = (gb % 4) * 128
                        ps = mtile()
                        for wslab, first in ((wf, True), (rf, False)):
                            for i in range(NI):
                                nc.tensor.matmul(
                                    ps[:],
                                    wslab[:, 2 * i:2 * i + 2, gl:gl + 128],
                                    mt[:, 2 * i:2 * i + 2, :],
                                    start=(first and i == 0),
                                    stop=(not first and i == NI - 1),
                                    perf_mode=DR)
                        nc.scalar.activation(
                            gt[:, gb, :], ps[:], AF.Gelu_apprx_tanh,
                            bias=bfc[:, gb:gb + 1], scale=inv_w)
                return emit

            yo_tiles = {}
            mslab = {}

            def mslab_prefetch(fh):
                # issue the slab DMA well before the first mproj piece needs
                # it (the DMA still waits on the previous slab's last reader)
                def emit():
                    mslab[fh] = load_mslab(fh)
                return emit

            def mproj_piece(tb, gi, fh):
                def emit():
                    tbl = slice((tb - 4 * gi) * 128, (tb - 4 * gi + 1) * 128)
                    NG = c.GB // 2
                    wm, rm = mslab[fh]
                    yo = yop.tile([128, 512], F32, tag="yo",
                                  name=f"yo{tb}_{fh}")
                    fsl = slice(fh * 512, (fh + 1) * 512)
                    ps = mtile()
                    for wslab, first in ((wm, True), (rm, False)):
                        for g in range(NG):
                            nc.tensor.matmul(
                                ps[:], gt[:, 2 * g:2 * g + 2, tbl],
                                wslab[:, 2 * g:2 * g + 2, :],
                                start=(first and g == 0),
                                stop=False, perf_mode=DR)
                    nc.tensor.matmul(ps[:], onesb[:], bmp_row[0:1, fsl],
                                     start=False, stop=True)
                    nc.vector.tensor_add(yo[:], ps[:], xloc[tb][:, fsl])
                    nc.sync.dma_start(
                        out=y_out[tb * 128:(tb + 1) * 128, fsl], in_=yo[:])
                return emit

            # ---- attention for one (jj, gi) block ----
            groups = [list(range(c.SPG * gi, c.SPG * (gi + 1)))
                      for gi in range(c.SLOTS // c.SPG)]

            def attention(jj, gi):
                # QK/exp run LAG plain-steps ahead of AV so the AV of the
                # next block never stalls the in-order PE queue on the
                # previous block's softmax-normalize reads of `pos`.
                LAG = 2
                g = groups[gi]
                s0, s3 = g[0], g[-1]
                n_loc = (s3 + 1) * c.CPB
                pos = pso.tile([65, 2, 512], F32, tag="po", name="po")
                pending = deque()

                def emit_av(item):
                    loc, hp, h, lo, pt, w = item
                    nc.tensor.matmul(
                        pos[:, hp, (lo - s0) * c.BS:512],
                        vtt[:, :, loc, h, :], pt[:, :, 0:w],
                        start=(loc == 0), stop=(loc == n_loc - 1),
                        perf_mode=DR)

                for loc in range(n_loc):
                    lo = max(s0, loc // c.CPB)
                    w = (s3 - lo + 1) * c.BS
                    qsl = slice(lo * c.BS, (s3 + 1) * c.BS)
                    diag = loc // c.CPB >= s0
                    for hp in range(c.HPB):
                        h = c.HPB * jj + hp
                        base = hp * 64
                        ps2 = psqk.tile([128, 2, 512], F32, tag="qk")
                        pt = ptp.tile([128, 2, 512], F8, tag="pt")
                        for ix in range(2):
                            kc = loc + ix * c.KCH
                            nc.tensor.matmul(
                                ps2[:, ix, 0:w],
                                ktp[base:base + 64, jj,
                                    kc * 128:(kc + 1) * 128],
                                qtp[base:base + 64, jj, qsl],
                                start=True, stop=not diag)
                            if diag:
                                nc.tensor.matmul(
                                    ps2[:, ix, 0:c.BS], identb[:],
                                    masks[:, kc, :],
                                    start=False, stop=True)
                        nc.scalar.activation(pt[:, :, 0:w], ps2[:, :, 0:w],
                                             AF.Exp, scale=1.0 / (KQS * KQS))
                        pending.append((loc, hp, h, lo, pt, w))
                        while len(pending) > LAG * c.HPB:
                            emit_av(pending.popleft())
                while pending:
                    emit_av(pending.popleft())
                for hp in range(c.HPB):
                    base = hp * 64
                    rec = recp.tile([1, 512], F32, tag="rec")
                    nc.vector.reciprocal(rec[:], pos[64:65, hp, :])
                    bcs = recp.tile([64, 512], F32, tag="bcs")
                    nc.gpsimd.partition_broadcast(bcs[:], rec[:])
                    nc.vector.tensor_tensor(
                        out=at2[gi][base:base + 64, jj, :],
                        in0=pos[0:64, hp, :], in1=bcs[:], op=ALU.mult)

            # ---- the interleaved emission ----
            filler = deque(kq_piece(m) for m in range(c.DC))
            for gi in range(2):
                for jj in range(c.DC):
                    # keep the K/Q chunk needed by the NEXT jj ahead of
                    # its attention block during gi=0
                    while (gi == 0 and filler
                           and len(filler) > (c.DC - 2 - jj)):
                        filler.popleft()()
                    attention(jj, gi)
                    if gi == 1:
                        for _ in range(2):
                            if filler:
                                filler.popleft()()
                while gi == 0 and filler:
                    filler.popleft()()
                if gi == 0:
                    es_wa.close()
                    es_ht.close()
                # queue this half's MLP chain as filler for the next half.
                # mproj fh-order alternates per half so the still-resident
                # slab pair is reused (3 slab loads total instead of 4)
                for tb in range(4 * gi, 4 * gi + 4):
                    filler.append(cproj_piece(tb, gi))
                filler.append(ln2_piece(gi))
                fhs = (0, 1) if gi == 0 else (1, 0)
                filler.append(mslab_prefetch(fhs[0]) if gi == 0
                              else fc_piece(0, gi))
                filler.append(fc_piece(0, gi) if gi == 0
                              else fc_piece(16, gi))
                if gi == 0:
                    filler.append(fc_piece(16, gi))
                for tb in range(4 * gi, 4 * gi + 4):
                    filler.append(mproj_piece(tb, gi, fhs[0]))
                filler.append(mslab_prefetch(fhs[1]))
                for tb in range(4 * gi, 4 * gi + 4):
                    filler.append(mproj_piece(tb, gi, fhs[1]))
            while filler:
                filler.popleft()()
            es_fc.close()

    nc.compile()
    return nc


def core_rows(cfg, half):
    """absolute sequence rows owned by a core with parity half"""
    c = cfg
    loc = np.arange(c.T)
    return (2 * (loc // c.BS) + half) * c.BS + loc % c.BS


def make_core_inputs(cfg: Cfg, x, ln1_w, ln1_b, W_attn, b_attn, W_cproj,
                     b_cproj, ln2_w, ln2_b, W_fc, b_fc, W_mproj, b_mproj):
    """Split full inputs into one in_map per core."""
    c = cfg
    f32 = np.float32
    f8 = ml_dtypes.float8_e4m3fn
    bf = ml_dtypes.bfloat16

    def q8w(w):
        return np.ascontiguousarray(np.asarray(w, f32) * WS).astype(f8)

    # fold LN1 affine + query scale into W_attn / b_attn
    ln1_w = np.asarray(ln1_w, f32)
    ln1_b = np.asarray(ln1_b, f32)
    Wa = np.asarray(W_attn, f32) * ln1_w[:, None]
    ba = np.asarray(b_attn, f32) + ln1_b @ np.asarray(W_attn, f32)
    qs = 1.0 / math.sqrt(c.HD)
    Wa = Wa.copy()
    Wa[:, :c.D] *= qs
    ba = ba.copy()
    ba[:c.D] *= qs

    # fold LN2 affine into W_fc / b_fc
    ln2_w = np.asarray(ln2_w, f32)
    ln2_b = np.asarray(ln2_b, f32)
    Wf = np.asarray(W_fc, f32) * ln2_w[:, None]
    bf_fold = np.asarray(b_fc, f32) + ln2_b @ np.asarray(W_fc, f32)

    # fold the V bias through the softmax (weights sum to 1): it becomes a
    # constant shift of the attention output -> bv @ W_cproj joins b_cproj
    bv = ba[2 * c.D:3 * c.D]
    bcp_tot = np.asarray(b_cproj, f32) + bv @ np.asarray(W_cproj, f32)

    Wf8 = q8w(Wf)
    Rf8 = np.ascontiguousarray(
        np.asarray(Wf, f32) * WS - np.asarray(Wf8, f32)).astype(f8)
    Wm8 = q8w(W_mproj)
    Rm8 = np.ascontiguousarray(
        np.asarray(W_mproj, f32) * WS - np.asarray(Wm8, f32)).astype(f8)

    shared = {
        "w_attn": q8w(Wa),
        "w_cproj": q8w(W_cproj),
        "w_fc": Wf8,
        "r_fc": Rf8,
        "w_mproj": Wm8,
        "r_mproj": Rm8,
        "bcp": np.ascontiguousarray(
            TRUNK * bcp_tot).astype(bf).reshape(1, c.D),
        "bmp": np.ascontiguousarray(
            TRUNK * np.asarray(b_mproj, f32)).astype(bf).reshape(1, c.D),
        "bfc": np.ascontiguousarray(bf_fold.reshape(c.GB, 128).T),
        # k/q evictions: out = psum/(WS/KQS) + KQS*bias
        "battn_qk": np.ascontiguousarray(
            KQS * ba[:2 * c.D].reshape(2 * c.DC, 128).T),
    }

    x = np.asarray(x, f32)
    in_maps = []
    for core in range(c.n_cores):
        b, half = core // 2, core % 2
        own = core_rows(c, half)
        peer = core_rows(c, 1 - half)
        perm = np.concatenate([own, peer])
        m = dict(shared)
        m["x"] = np.ascontiguousarray(TRUNK * x[b][perm])
        m["qidx"] = own.astype(f32).reshape(1, c.T)
        kofs = np.empty((128, c.KC), f32)
        for kc in range(c.KC):
            kofs[:, kc] = perm[kc * 128 + np.arange(128)]
        m["kofs"] = kofs
        in_maps.append(m)
    return in_maps


_NC_CACHE = {}


def get_nc(cfg: Cfg):
    key = (cfg.B, cfg.S, cfg.D, cfg.H, cfg.F, cfg.BS)
    if key not in _NC_CACHE:
        _NC_CACHE[key] = build(cfg)
    return _NC_CACHE[key]


def kernel(**inputs) -> np.ndarray:
    from concourse.bass_utils import run_bass_kernel_spmd

    cfg = Cfg()
    nc = get_nc(cfg)
    in_maps = make_core_inputs(cfg, **inputs)
    res = run_bass_kernel_spmd(nc, in_maps, core_ids=list(range(cfg.n_cores)))
    B, S, D = cfg.B, cfg.S, cfg.D
    out = np.empty((B, S, D), np.float32)
    inv_trunk = 1.0 / TRUNK
    for core in range(cfg.n_cores):
        b, half = core // 2, core % 2
        out[b, core_rows(cfg, half), :] = inv_trunk * res.results[core]["y"]
    return out
